# revision 20
# baseline (speedup 1.0000x reference)
"""Trainium2 Bass kernel for a single-head causal attention block (bf16).

Reference computation (B=4, T=2048, C=1024, H=64):
    q = x @ Wq; k = x @ Wk; v = x @ Wv          # [B,T,H]
    scores = (q @ k^T) * C**-0.5                # causal masked
    out = softmax(scores) @ v                   # [B,T,H]

Sharding: 2 cores per batch (8 cores, B=4). Core (b, t) owns the 4
interleaved 256-row query chunks {t, t+2, t+4, t+6} of batch b, which
balances causal work exactly across the pair. One uniform SPMD program;
all per-core differences are input data (row arrangement + 0/1 masks).

v4 design (vs v2/v3):
  * x^T is prepared on the host (numpy) — both halves arrive via plain
    1 MB HWDGE DMAs instead of xbar transpose-DMAs.
  * Two-stage software pipeline with explicit double-buffered tile sets:
    each loop-body instance runs front(n+1) = {const+x loads, QKV
    projections, pairwise KV exchange, kvv gather} and then attn(n) on
    the PREVIOUS instance's set.  The whole exchange chain of iteration
    n+1 hides under the ~10us attention phase of iteration n, so the PE
    never stalls between projections and attention.
  * A prologue front() before the loop fills the first set; an epilogue
    attn() after the loop drains the last.  Benchmark slope timing is
    unaffected (constant offset).
  * Exchange chains split across queues: SP carries half-0 (x^T h0
    load, in_cc0 store, kvv0 gather, y even), ACT carries half-1.
  * KV/Q PSUM->SBUF copies on DVE; ScalarE does only exp in steady
    state.  V' ones/zeros columns are memset once at setup.
  * PE HAM warmup matmuls only in one-shot mode; the steady-state loop
    keeps the PE clock hot by itself.

Per-core attention (unchanged from v2): q is projected with
column-duplicated weights so q^T exists on partitions 0:64 and 64:128;
K^T is interleaved even/odd-chunk on partition halves so score matmuls
run as concurrent 64-contraction PE row tiles; exp is one N=1024
ScalarE activation per 4-unit group; PV accumulates [V|1|0]^T @ es.
"""

import contextlib

import numpy as np

B, T, C, H = 4, 2048, 1024, 64
NCORES = 8
P = 128          # partitions
NCB = C // P     # 8 channel blocks
QB = 256         # query block width
SCALE = float(C) ** -0.5

_CACHE = {}


def _build_program(loop_n=1, fake_cc=False, flat=False, unroll=4,
                   nodma=False):
    # fake_cc: replace the AllGather with equivalent-volume local DMAs —
    # numerically wrong (peer half duplicated) but timing-equivalent; used
    # only by the benchmark loop, where real collectives desync.
    # flat=True: emit loop_n sequential instances with no For_i (sim-able).
    import concourse.bacc as bacc
    import concourse.mybir as mybir
    from concourse import tile

    f32 = mybir.dt.float32
    bf16 = mybir.dt.bfloat16
    EXP = mybir.ActivationFunctionType.Exp

    nc = bacc.Bacc("TRN2", target_bir_lowering=False, debug=False,
                   num_devices=NCORES)

    xqT_d = nc.dram_tensor("xqT", [P, 2, NCB, 512], bf16,
                           kind="ExternalInput").ap()
    wq2_d = nc.dram_tensor("wq2", [P, NCB, P], bf16, kind="ExternalInput").ap()
    wkv_d = nc.dram_tensor("wkv", [P, NCB, P], bf16, kind="ExternalInput").ap()
    iden_d = nc.dram_tensor("iden", [P, P], bf16, kind="ExternalInput").ap()
    mask_d = nc.dram_tensor("mask", [P, 4, QB], bf16, kind="ExternalInput").ap()
    y_d = nc.dram_tensor("y", [T // 2, H], bf16, kind="ExternalOutput").ap()

    one_shot = (loop_n == 1)
    U = 1 if one_shot else (loop_n if flat else unroll)
    trip = 1 if (one_shot or flat) else loop_n // U
    assert one_shot or (U % 2 == 0 and (flat or loop_n % U == 0))
    NSET = 1 if one_shot else 2

    hw_eng = [nc.sync, nc.scalar]
    ctr = [0]

    def _nm(base):
        ctr[0] += 1
        return f"{base}_{ctr[0]}"

    with tile.TileContext(nc) as tc:
        with (
            tc.tile_pool(name="sets", bufs=1) as setp,
            tc.tile_pool(name="exps", bufs=3) as expp,
            tc.tile_pool(name="small", bufs=4) as smallp,
            tc.tile_pool(name="pt", bufs=2, space="PSUM") as psum_t,
            tc.tile_pool(name="psc", bufs=2, space="PSUM") as psum_sc,
            tc.tile_pool(name="po", bufs=2, space="PSUM") as psum_o,
            tc.tile_pool(name="dram", bufs=1, space="DRAM") as dramp,
        ):
            # ---- static one-time tiles ----
            zbias = setp.tile([P, 1], f32, name="zbias")
            nc.vector.memset(zbias[:], 0.0)
            # warm the ACT exp table-set early (one-time table DMA load
            # otherwise lands on the attention critical path)
            expwarm = setp.tile([P, 1], f32, name="expwarm")
            nc.scalar.activation(expwarm[:], zbias[:], EXP, bias=zbias[:])
            # mask and identity are kernel-internal constants: load once
            mask_g = setp.tile([P, 4, QB], bf16, name="mask_g")
            nc.gpsimd.dma_start(mask_g[:], mask_d)
            iden_g = setp.tile([P, P], bf16, name="iden_g")
            nc.gpsimd.dma_start(iden_g[:], iden_d)

            # ---- double-buffered pipeline sets ----
            def make_set(s):
                S = {}
                S["wkv"] = setp.tile([P, NCB, P], bf16, name=f"wkv{s}")
                S["wq2"] = setp.tile([P, NCB, P], bf16, name=f"wq2{s}")
                S["xT"] = [setp.tile([P, NCB, 512], bf16, name=f"xT{h}_{s}")
                           for h in range(2)]
                S["kvo"] = [setp.tile([P, 512], bf16, name=f"kvo{h}_{s}")
                            for h in range(2)]
                S["qT"] = [setp.tile([P, 512], bf16, name=f"qT{h}_{s}")
                           for h in range(2)]
                S["incc"] = [dramp.tile([P, 512], bf16, name=f"incc{h}_{s}")
                             for h in range(2)]
                S["outcc"] = [dramp.tile([2 * P, 512], bf16,
                                         name=f"outcc{h}_{s}")
                              for h in range(2)]
                # kvv: cols 0:512 K^T units, 512:1024 V^T units; rows 0:64
                # = even-chunk units, rows 64:128 = odd-chunk units
                S["kvv"] = [setp.tile([P, 1024], bf16, name=f"kvv{h}_{s}")
                            for h in range(2)]
                # V' = [V | 1 | 0] per s-unit: vp[h][:, u, parity, 66]
                S["vp"] = [setp.tile([P, 4, 2, H + 2], bf16,
                                     name=f"vp{h}_{s}")
                           for h in range(2)]
                for h in range(2):
                    nc.vector.memset(S["vp"][h][:, :, :, H:H + 1], 1.0)
                    nc.vector.memset(S["vp"][h][:, :, :, H + 1:H + 2], 0.0)
                S["ys"] = setp.tile([P, 8, H], bf16, name=f"ys{s}")
                if nodma:
                    for t in (S["wkv"], S["wq2"],
                              S["xT"][0], S["xT"][1], S["kvv"][0],
                              S["kvv"][1], S["qT"][0], S["qT"][1]):
                        nc.vector.memset(t[:], 0.125)
                return S

            sets = [make_set(s) for s in range(NSET)]

            if one_shot:
                # warm the PE HAM clock gate during the x-load window so
                # the projections run at 2.4 GHz
                dummy = setp.tile([P, 512], bf16, name="dummy")
                nc.vector.memset(dummy[:], 0.0)
                pwarm = psum_t.tile([P, 512], f32, tag="pt", name="pwarm")
                for w in range(8):
                    nc.tensor.matmul(pwarm[:], dummy[:, 0:P], dummy[:],
                                     start=(w == 0), stop=(w == 7))

            def emit_prefetch_early(S):
                # weights + x for iteration n+2: issued at body start so the
                # 1 MB x transfers and the weight loads complete a full body
                # before their projections need them; weights ride the two
                # HWDGE queues behind the x halves (SWDGE keeps only the
                # collective traffic)
                if nodma:
                    return
                for h in range(2):
                    hw_eng[h].dma_start(S["xT"][h][:, 0:4], xqT_d[:, h, 0:4])
                    hw_eng[h].dma_start(S["xT"][h][:, 4:8], xqT_d[:, h, 4:8])
                nc.sync.dma_start(S["wkv"][:], wkv_d)
                nc.scalar.dma_start(S["wq2"][:], wq2_d)

            def emit_front_loads(S):
                emit_prefetch_early(S)

            def make_proj_chunks(S):
                # 8 PE chunks of 4 projection matmuls each, interleaved by
                # emit_attn into the previous iteration's attention groups:
                # PE fills exp-wait bubbles with next-iteration projections
                # and the KV exchange hides under the attention phase.
                st = {}

                def proj_part(w_s, ps_key, h, lo):
                    if lo == 0:
                        st[ps_key] = psum_t.tile([P, 512], f32, tag="pt",
                                                 name=_nm(ps_key))
                    pp = st[ps_key]
                    for cb in range(lo, lo + 4):
                        nc.tensor.matmul(pp[:], w_s[:, cb, :],
                                         S["xT"][h][:, cb, :],
                                         start=(cb == 0),
                                         stop=(cb == NCB - 1))

                def kv_done(h):
                    nc.scalar.copy(S["kvo"][h][:], st[f"pkv{h}"][:])
                    if nodma:
                        return
                    hw_eng[h].dma_start(S["incc"][h][:], S["kvo"][h][:])
                    if fake_cc:
                        nc.gpsimd.dma_start(S["outcc"][h][0:P, :],
                                            S["incc"][h][:])
                        nc.gpsimd.dma_start(S["outcc"][h][P:2 * P, :],
                                            S["incc"][h][:])
                    else:
                        nc.gpsimd.collective_compute(
                            "AllGather",
                            mybir.AluOpType.bypass,
                            replica_groups=[[2 * b, 2 * b + 1]
                                            for b in range(NCORES // 2)],
                            ins=[S["incc"][h].opt()],
                            outs=[S["outcc"][h].opt()],
                        )
                    # kvv src rows (a, x): a = t-core, x = K/V row
                    src = S["outcc"][h][:].rearrange("(a x) c -> a x c", a=2)
                    for kv in range(2):
                        hw_eng[h].dma_start(
                            S["kvv"][h][:, kv * 512:(kv + 1) * 512],
                            src[:, kv * H:(kv + 1) * H, :])

                def q_done(h):
                    nc.scalar.copy(S["qT"][h][:], st[f"pq{h}"][:])

                chunks = []
                for h in range(2):
                    chunks.append(lambda h=h: proj_part(S["wkv"], f"pkv{h}",
                                                        h, 0))
                    chunks.append(lambda h=h: (proj_part(S["wkv"], f"pkv{h}",
                                                         h, 4), kv_done(h)))
                    chunks.append(lambda h=h: proj_part(S["wq2"], f"pq{h}",
                                                        h, 0))
                    chunks.append(lambda h=h: (proj_part(S["wq2"], f"pq{h}",
                                                         h, 4), q_done(h)))
                return chunks

            def emit_front(S):
                # front-compute: projections + exchange (inputs prefetched)
                for c in make_proj_chunks(S):
                    c()

            def emit_vprime(S, h):
                # one [128,128] transpose of a V^T column block yields
                # V natural for the even unit AND the odd unit at once
                for u in range(4):
                    pvv = psum_t.tile([P, P], bf16, tag="pt", name=_nm("pvv"))
                    nc.tensor.transpose(
                        pvv[:],
                        S["kvv"][h][:, 512 + u * P:512 + (u + 1) * P],
                        iden_g[:])
                    nc.vector.tensor_copy(
                        S["vp"][h][:, u, :, 0:H],
                        pvv[:].rearrange("p (a c) -> p a c", a=2))

            def emit_attn(S, interleave=(), vprime0_done=False):
                interleave = list(interleave)
                kvv, qT2, vp, ys = (S["kvv"], S["qT"], S["vp"], S["ys"])
                mask_s, iden = mask_g, iden_g

                if not vprime0_done:
                    emit_vprime(S, 0)

                # Group (i, g) covers unit-pairs {2g, 2g+1}; pair j = even
                # unit j (rows 0:64) + odd unit j (rows 64:128), run as
                # concurrent PE row-tiles.  psum cols:
                # [ev 2g | ev 2g+1 | od 2g | od 2g+1].  Order: groups
                # needing only half-0 kv first; (3,3) before (3,2) so the
                # final group has no mask work in the tail.
                pairs = [(0, 0), (1, 0), (1, 1), (2, 0), (2, 1),
                         (3, 0), (3, 1), (2, 2), (3, 3), (3, 2)]
                es_t = {}
                po_t = {}
                ot_t = {}

                def emit_scores(p):
                    i, g = pairs[p]
                    ps = psum_sc.tile([P, 1024], f32, tag="ps", name=_nm("ps"))
                    for k in range(2):
                        j = 2 * g + k
                        co = (j % 4) * P
                        qs = slice((i % 2) * QB, (i % 2 + 1) * QB)
                        nc.tensor.matmul(
                            ps[:, k * QB:(k + 1) * QB],
                            kvv[j // 4][0:H, co:co + P],
                            qT2[i // 2][0:H, qs], start=True, stop=True)
                        nc.tensor.matmul(
                            ps[:, 512 + k * QB:512 + (k + 1) * QB],
                            kvv[j // 4][H:P, co:co + P],
                            qT2[i // 2][H:P, qs], start=True, stop=True)
                    es = expp.tile([P, 1024], bf16, tag="es", name=_nm("es"))
                    nc.scalar.activation(es[:], ps[:], EXP,
                                         bias=zbias[:], scale=SCALE)
                    if g == i:  # diagonal group: mask last even+odd pairs
                        for k in range(2):
                            esl = es[:, k * QB:(k + 1) * QB]
                            nc.vector.tensor_mul(esl, esl, mask_s[:, k, :])
                            osl = es[:, 512 + k * QB:512 + (k + 1) * QB]
                            nc.vector.tensor_mul(osl, osl,
                                                 mask_s[:, 2 + k, :])
                    es_t[p] = es

                first_p = {}
                last_p = {}
                for p, (i, g) in enumerate(pairs):
                    first_p.setdefault(i, p)
                    last_p[i] = p

                def emit_pv(p):
                    i, g = pairs[p]
                    if p == first_p[i]:
                        po_t[i] = psum_o.tile([H + 2, QB], f32, tag="po", name=_nm("po"))
                    es = es_t.pop(p)
                    for k in range(2):
                        j = 2 * g + k
                        nc.tensor.matmul(
                            po_t[i][:], vp[j // 4][:, j % 4, 0, 0:H + 2],
                            es[:, k * QB:(k + 1) * QB],
                            start=(p == first_p[i] and k == 0), stop=False)
                        nc.tensor.matmul(
                            po_t[i][:], vp[j // 4][:, j % 4, 1, 0:H + 2],
                            es[:, 512 + k * QB:512 + (k + 1) * QB],
                            start=False, stop=(p == last_p[i] and k == 1))
                    if p == last_p[i]:
                        po = po_t.pop(i)
                        ot = smallp.tile([H + 2, QB], bf16, tag="ot", name=_nm("ot"))
                        nc.vector.tensor_copy(ot[:], po[:])
                        ot_t[i] = ot

                def emit_out(i):
                    # transpose out^T back, divide by the denominator
                    # column, store
                    ot = ot_t.pop(i)
                    for h2 in range(2):
                        pt2 = psum_t.tile([P, H + 2], bf16, tag="pt", name=_nm("pt2"))
                        nc.tensor.transpose(
                            pt2[:], ot[0:H + 2, h2 * P:(h2 + 1) * P],
                            iden[0:H + 2, 0:H + 2])
                        rc = smallp.tile([P, 1], f32, tag="rc", name=_nm("rc"))
                        nc.vector.reciprocal(rc[:], pt2[:, H:H + 1])
                        nc.vector.tensor_scalar_mul(ys[:, 2 * i + h2, :],
                                                    pt2[:, 0:H], rc[:])
                    if i % 2 == 1 and not nodma:
                        u0 = (i - 1) * 2
                        dst = y_d[u0 * P:(u0 + 4) * P, :].rearrange(
                            "(u p) c -> p u c", u=4)
                        nc.gpsimd.dma_start(dst, ys[:, u0:u0 + 4, :])

                # scores run 2 groups ahead of PV so the ScalarE exp (and
                # the DVE mask multiplies) never gate a PV matmul
                LA = 2
                for p in range(len(pairs)):
                    if p == 7:
                        emit_vprime(S, 1)
                    emit_scores(p)
                    if p >= LA:
                        emit_pv(p - LA)
                        if p - LA == last_p[pairs[p - LA][0]]:
                            emit_out(pairs[p - LA][0])
                for p in range(len(pairs) - LA, len(pairs)):
                    emit_pv(p)
                    if p == last_p[pairs[p][0]]:
                        emit_out(pairs[p][0])
                for c in interleave:
                    c()

            # ---- 3-deep pipeline ----
            # body u: [prefetch(u+2) early | front-compute(u+1) |
            #          attn(u) | prefetch(u+2) late]; loads lead their
            # consumers by two bodies, projections+exchange by one.
            import concourse.mybir as mybir_
            if one_shot:
                emit_front_loads(sets[0])
                emit_front(sets[0])
                emit_attn(sets[0])
            else:
                emit_front_loads(sets[0])  # iter 0 inputs
                emit_front_loads(sets[1])  # iter 1 inputs
                emit_front(sets[0])        # front-compute(0)
                with (tc.For_i(0, trip, 1,
                               hint_engines=(mybir_.EngineType.PE,
                                             mybir_.EngineType.SP,
                                             mybir_.EngineType.Activation,
                                             mybir_.EngineType.DVE,
                                             mybir_.EngineType.Pool))
                      if trip > 1 else contextlib.nullcontext()):
                    for u in range(U):
                        emit_prefetch_early(sets[u % NSET])
                        emit_vprime(sets[u % NSET], 0)
                        emit_front(sets[(u + 1) % NSET])
                        emit_attn(sets[u % NSET], vprime0_done=True)
                emit_attn(sets[(U - 1) % NSET])

    nc.compile()
    return nc


def _make_masks():
    i = np.arange(P)[:, None]
    j = np.arange(QB)[None, :]
    ma = (i <= j).astype(np.float32)
    mb = (i + P <= j).astype(np.float32)
    return ma, mb


def make_in_maps(x, Wq, Wk, Wv):
    """Per-core input dicts. Core 2*b + t owns query chunks {t, t+2, t+4, t+6}.

    kvv layout after the pairwise AllGather is global-fixed: even-chunk
    K^T units on partitions 0:64, odd-chunk on 64:128; q-block i masks
    its last even pair (t=0: diagonal, t=1: ones) and last odd pair
    (t=0: zeros, t=1: diagonal).
    """
    import ml_dtypes
    bf16 = ml_dtypes.bfloat16

    wkv = np.concatenate([Wk, Wv], axis=1).astype(np.float32)
    wkv = np.ascontiguousarray(
        wkv.reshape(NCB, P, P).transpose(1, 0, 2)).astype(bf16)
    wq = np.asarray(Wq, np.float32).reshape(NCB, P, H).transpose(1, 0, 2)
    wq2 = np.ascontiguousarray(
        np.concatenate([wq, wq], axis=2)).astype(bf16)
    iden = np.eye(P, dtype=np.float32).astype(bf16)
    ma, mb = _make_masks()
    ones = np.ones((P, QB), np.float32)
    zeros = np.zeros((P, QB), np.float32)
    xc = np.asarray(x, np.float32).reshape(B, 8, QB, C)
    in_maps = []
    for core in range(NCORES):
        b, t = divmod(core, 2)
        own = [2 * k + t for k in range(4)]
        xq = xc[b, own].reshape(T // 2, C)
        # host-side transpose: xqT[p, h, cb, t'] = xq[h*512+t', cb*128+p]
        xqT = np.ascontiguousarray(
            xq.T.reshape(NCB, P, 2, 512).transpose(1, 2, 0, 3)).astype(bf16)
        if t == 0:
            msk = np.stack([ma, mb, zeros, zeros], axis=1)
        else:
            msk = np.stack([ones, ones, ma, mb], axis=1)
        in_maps.append({
            "xqT": xqT, "wq2": wq2, "wkv": wkv, "iden": iden,
            "mask": np.ascontiguousarray(msk).astype(bf16),
        })
    return in_maps


def assemble(results):
    y = np.empty((B, T, H), np.float32)
    for core in range(NCORES):
        b, t = divmod(core, 2)
        yc = results[core]["y"]
        for i in range(4):
            g = 2 * i + t
            y[b, g * QB:(g + 1) * QB, :] = yc[i * QB:(i + 1) * QB, :]
    return y


def kernel(x, Wq, Wk, Wv):
    from concourse.bass_utils import run_bass_kernel_spmd
    if "nc" not in _CACHE:
        _CACHE["nc"] = _build_program()
    nc = _CACHE["nc"]
    in_maps = make_in_maps(x, Wq, Wk, Wv)
    try:
        res = run_bass_kernel_spmd(nc, in_maps, list(range(NCORES)))
    except Exception:
        # transient NRT device errors on a cold first dispatch recover on
        # retry
        res = run_bass_kernel_spmd(nc, in_maps, list(range(NCORES)))
    return assemble(res.results)


# revision 21
# speedup vs baseline: 1.0563x; 1.0563x over previous
"""Trainium2 Bass kernel for a single-head causal attention block (bf16).

Reference computation (B=4, T=2048, C=1024, H=64):
    q = x @ Wq; k = x @ Wk; v = x @ Wv          # [B,T,H]
    scores = (q @ k^T) * C**-0.5                # causal masked
    out = softmax(scores) @ v                   # [B,T,H]

Sharding: 2 cores per batch (8 cores, B=4). Core (b, t) owns the 4
interleaved 256-row query chunks {t, t+2, t+4, t+6} of batch b, which
balances causal work exactly across the pair. One uniform SPMD program;
all per-core differences are input data (row arrangement + 0/1 masks).

v4 design (vs v2/v3):
  * x^T is prepared on the host (numpy) — both halves arrive via plain
    1 MB HWDGE DMAs instead of xbar transpose-DMAs.
  * Two-stage software pipeline with explicit double-buffered tile sets:
    each loop-body instance runs front(n+1) = {const+x loads, QKV
    projections, pairwise KV exchange, kvv gather} and then attn(n) on
    the PREVIOUS instance's set.  The whole exchange chain of iteration
    n+1 hides under the ~10us attention phase of iteration n, so the PE
    never stalls between projections and attention.
  * A prologue front() before the loop fills the first set; an epilogue
    attn() after the loop drains the last.  Benchmark slope timing is
    unaffected (constant offset).
  * Exchange chains split across queues: SP carries half-0 (x^T h0
    load, in_cc0 store, kvv0 gather, y even), ACT carries half-1.
  * KV/Q PSUM->SBUF copies on DVE; ScalarE does only exp in steady
    state.  V' ones/zeros columns are memset once at setup.
  * PE HAM warmup matmuls only in one-shot mode; the steady-state loop
    keeps the PE clock hot by itself.

Per-core attention (unchanged from v2): q is projected with
column-duplicated weights so q^T exists on partitions 0:64 and 64:128;
K^T is interleaved even/odd-chunk on partition halves so score matmuls
run as concurrent 64-contraction PE row tiles; exp is one N=1024
ScalarE activation per 4-unit group; PV accumulates [V|1|0]^T @ es.
"""

import contextlib

import numpy as np

B, T, C, H = 4, 2048, 1024, 64
NCORES = 8
P = 128          # partitions
NCB = C // P     # 8 channel blocks
QB = 256         # query block width
SCALE = float(C) ** -0.5

_CACHE = {}


def _build_program(loop_n=1, fake_cc=False, flat=False, unroll=4,
                   nodma=False):
    # fake_cc: replace the AllGather with equivalent-volume local DMAs —
    # numerically wrong (peer half duplicated) but timing-equivalent; used
    # only by the benchmark loop, where real collectives desync.
    # flat=True: emit loop_n sequential instances with no For_i (sim-able).
    import concourse.bacc as bacc
    import concourse.mybir as mybir
    from concourse import tile

    f32 = mybir.dt.float32
    bf16 = mybir.dt.bfloat16
    EXP = mybir.ActivationFunctionType.Exp

    nc = bacc.Bacc("TRN2", target_bir_lowering=False, debug=False,
                   num_devices=NCORES)

    xqT_d = nc.dram_tensor("xqT", [P, 2, NCB, 512], bf16,
                           kind="ExternalInput").ap()
    wq2_d = nc.dram_tensor("wq2", [P, NCB, P], bf16, kind="ExternalInput").ap()
    wkv_d = nc.dram_tensor("wkv", [P, NCB, P], bf16, kind="ExternalInput").ap()
    iden_d = nc.dram_tensor("iden", [P, P], bf16, kind="ExternalInput").ap()
    mask_d = nc.dram_tensor("mask", [P, 4, QB], bf16, kind="ExternalInput").ap()
    y_d = nc.dram_tensor("y", [T // 2, H], bf16, kind="ExternalOutput").ap()

    one_shot = (loop_n == 1)
    U = 1 if one_shot else (loop_n if flat else unroll)
    trip = 1 if (one_shot or flat) else loop_n // U
    assert one_shot or (U % 2 == 0 and (flat or loop_n % U == 0))
    NSET = 1 if one_shot else 2

    hw_eng = [nc.sync, nc.scalar]
    ctr = [0]

    def _nm(base):
        ctr[0] += 1
        return f"{base}_{ctr[0]}"

    with tile.TileContext(nc) as tc:
        with (
            tc.tile_pool(name="sets", bufs=1) as setp,
            tc.tile_pool(name="exps", bufs=3) as expp,
            tc.tile_pool(name="small", bufs=4) as smallp,
            tc.tile_pool(name="pt", bufs=2, space="PSUM") as psum_t,
            tc.tile_pool(name="psc", bufs=2, space="PSUM") as psum_sc,
            tc.tile_pool(name="po", bufs=2, space="PSUM") as psum_o,
            tc.tile_pool(name="dram", bufs=1, space="DRAM") as dramp,
        ):
            # ---- static one-time tiles ----
            zbias = setp.tile([P, 1], f32, name="zbias")
            nc.vector.memset(zbias[:], 0.0)
            # warm the ACT exp table-set early (one-time table DMA load
            # otherwise lands on the attention critical path)
            expwarm = setp.tile([P, 1], f32, name="expwarm")
            nc.scalar.activation(expwarm[:], zbias[:], EXP, bias=zbias[:])
            # mask and identity are kernel-internal constants: load once
            mask_g = setp.tile([P, 4, QB], bf16, name="mask_g")
            nc.gpsimd.dma_start(mask_g[:], mask_d)
            iden_g = setp.tile([P, P], bf16, name="iden_g")
            nc.gpsimd.dma_start(iden_g[:], iden_d)

            # ---- double-buffered pipeline sets ----
            def make_set(s):
                S = {}
                S["wkv"] = setp.tile([P, NCB, P], bf16, name=f"wkv{s}")
                S["wq2"] = setp.tile([P, NCB, P], bf16, name=f"wq2{s}")
                S["xT"] = [setp.tile([P, NCB, 512], bf16, name=f"xT{h}_{s}")
                           for h in range(2)]
                S["kvo"] = [setp.tile([P, 512], bf16, name=f"kvo{h}_{s}")
                            for h in range(2)]
                S["qT"] = [setp.tile([P, 512], bf16, name=f"qT{h}_{s}")
                           for h in range(2)]
                S["incc"] = [dramp.tile([P, 512], bf16, name=f"incc{h}_{s}")
                             for h in range(2)]
                S["outcc"] = [dramp.tile([2 * P, 512], bf16,
                                         name=f"outcc{h}_{s}")
                              for h in range(2)]
                # kvv: cols 0:512 K^T units, 512:1024 V^T units; rows 0:64
                # = even-chunk units, rows 64:128 = odd-chunk units
                S["kvv"] = [setp.tile([P, 1024], bf16, name=f"kvv{h}_{s}")
                            for h in range(2)]
                # V' = [V | 1 | 0] per s-unit: vp[h][:, u, parity, 66]
                S["vp"] = [setp.tile([P, 4, 2, H + 2], bf16,
                                     name=f"vp{h}_{s}")
                           for h in range(2)]
                for h in range(2):
                    nc.vector.memset(S["vp"][h][:, :, :, H:H + 1], 1.0)
                    nc.vector.memset(S["vp"][h][:, :, :, H + 1:H + 2], 0.0)
                S["ys"] = setp.tile([P, 8, H], bf16, name=f"ys{s}")
                if nodma:
                    for t in (S["wkv"], S["wq2"],
                              S["xT"][0], S["xT"][1], S["kvv"][0],
                              S["kvv"][1], S["qT"][0], S["qT"][1]):
                        nc.vector.memset(t[:], 0.125)
                return S

            sets = [make_set(s) for s in range(NSET)]

            if one_shot:
                # warm the PE HAM clock gate during the x-load window so
                # the projections run at 2.4 GHz
                dummy = setp.tile([P, 512], bf16, name="dummy")
                nc.vector.memset(dummy[:], 0.0)
                pwarm = psum_t.tile([P, 512], f32, tag="pt", name="pwarm")
                for w in range(8):
                    nc.tensor.matmul(pwarm[:], dummy[:, 0:P], dummy[:],
                                     start=(w == 0), stop=(w == 7))

            def emit_prefetch_early(S):
                # weights + x for iteration n+2: issued at body start so the
                # 1 MB x transfers and the weight loads complete a full body
                # before their projections need them; weights ride the two
                # HWDGE queues behind the x halves (SWDGE keeps only the
                # collective traffic)
                if nodma:
                    return
                for h in range(2):
                    hw_eng[h].dma_start(S["xT"][h][:, 0:4], xqT_d[:, h, 0:4])
                    hw_eng[h].dma_start(S["xT"][h][:, 4:8], xqT_d[:, h, 4:8])
                nc.sync.dma_start(S["wkv"][:], wkv_d)
                nc.scalar.dma_start(S["wq2"][:], wq2_d)

            def emit_front_loads(S):
                emit_prefetch_early(S)

            def make_proj_chunks(S):
                # 8 PE chunks of 4 projection matmuls each, interleaved by
                # emit_attn into the previous iteration's attention groups:
                # PE fills exp-wait bubbles with next-iteration projections
                # and the KV exchange hides under the attention phase.
                st = {}

                def proj_part(w_s, ps_key, h, lo):
                    if lo == 0:
                        st[ps_key] = psum_t.tile([P, 512], f32, tag="pt",
                                                 name=_nm(ps_key))
                    pp = st[ps_key]
                    for cb in range(lo, lo + 4):
                        nc.tensor.matmul(pp[:], w_s[:, cb, :],
                                         S["xT"][h][:, cb, :],
                                         start=(cb == 0),
                                         stop=(cb == NCB - 1))

                def kv_done(h):
                    nc.scalar.copy(S["kvo"][h][:], st[f"pkv{h}"][:])
                    if nodma:
                        return
                    hw_eng[h].dma_start(S["incc"][h][:], S["kvo"][h][:])
                    if fake_cc:
                        nc.gpsimd.dma_start(S["outcc"][h][0:P, :],
                                            S["incc"][h][:])
                        nc.gpsimd.dma_start(S["outcc"][h][P:2 * P, :],
                                            S["incc"][h][:])
                    else:
                        nc.gpsimd.collective_compute(
                            "AllGather",
                            mybir.AluOpType.bypass,
                            replica_groups=[[2 * b, 2 * b + 1]
                                            for b in range(NCORES // 2)],
                            ins=[S["incc"][h].opt()],
                            outs=[S["outcc"][h].opt()],
                        )
                    # kvv src rows (a, x): a = t-core, x = K/V row
                    src = S["outcc"][h][:].rearrange("(a x) c -> a x c", a=2)
                    for kv in range(2):
                        hw_eng[h].dma_start(
                            S["kvv"][h][:, kv * 512:(kv + 1) * 512],
                            src[:, kv * H:(kv + 1) * H, :])

                def q_done(h):
                    nc.scalar.copy(S["qT"][h][:], st[f"pq{h}"][:])

                chunks = []
                for h in range(2):
                    chunks.append(lambda h=h: proj_part(S["wkv"], f"pkv{h}",
                                                        h, 0))
                    chunks.append(lambda h=h: (proj_part(S["wkv"], f"pkv{h}",
                                                         h, 4), kv_done(h)))
                    chunks.append(lambda h=h: proj_part(S["wq2"], f"pq{h}",
                                                        h, 0))
                    chunks.append(lambda h=h: (proj_part(S["wq2"], f"pq{h}",
                                                         h, 4), q_done(h)))
                return chunks

            def emit_front(S):
                # front-compute: projections + exchange (inputs prefetched)
                for c in make_proj_chunks(S):
                    c()

            def emit_vprime(S, h):
                # one [128,128] transpose of a V^T column block yields
                # V natural for the even unit AND the odd unit at once
                for u in range(4):
                    pvv = psum_t.tile([P, P], bf16, tag="pt", name=_nm("pvv"))
                    nc.tensor.transpose(
                        pvv[:],
                        S["kvv"][h][:, 512 + u * P:512 + (u + 1) * P],
                        iden_g[:])
                    nc.vector.tensor_copy(
                        S["vp"][h][:, u, :, 0:H],
                        pvv[:].rearrange("p (a c) -> p a c", a=2))

            def emit_attn(S, interleave=(), vprime0_done=False):
                interleave = list(interleave)
                kvv, qT2, vp, ys = (S["kvv"], S["qT"], S["vp"], S["ys"])
                mask_s, iden = mask_g, iden_g

                if not vprime0_done:
                    emit_vprime(S, 0)

                # Group (i, g) covers unit-pairs {2g, 2g+1}; pair j = even
                # unit j (rows 0:64) + odd unit j (rows 64:128), run as
                # concurrent PE row-tiles.  psum cols:
                # [ev 2g | ev 2g+1 | od 2g | od 2g+1].  Order: groups
                # needing only half-0 kv first; (3,3) before (3,2) so the
                # final group has no mask work in the tail.
                pairs = [(0, 0), (1, 0), (1, 1), (2, 0), (2, 1),
                         (3, 0), (3, 1), (2, 2), (3, 3), (3, 2)]
                es_t = {}
                po_t = {}
                ot_t = {}

                def emit_scores(p):
                    i, g = pairs[p]
                    ps = psum_sc.tile([P, 1024], f32, tag="ps", name=_nm("ps"))
                    for k in range(2):
                        j = 2 * g + k
                        co = (j % 4) * P
                        qs = slice((i % 2) * QB, (i % 2 + 1) * QB)
                        nc.tensor.matmul(
                            ps[:, k * QB:(k + 1) * QB],
                            kvv[j // 4][0:H, co:co + P],
                            qT2[i // 2][0:H, qs], start=True, stop=True)
                        nc.tensor.matmul(
                            ps[:, 512 + k * QB:512 + (k + 1) * QB],
                            kvv[j // 4][H:P, co:co + P],
                            qT2[i // 2][H:P, qs], start=True, stop=True)
                    es = expp.tile([P, 1024], bf16, tag="es", name=_nm("es"))
                    nc.scalar.activation(es[:], ps[:], EXP,
                                         bias=zbias[:], scale=SCALE)
                    if g == i:  # diagonal group: mask last even+odd pairs
                        for k in range(2):
                            esl = es[:, k * QB:(k + 1) * QB]
                            nc.vector.tensor_mul(esl, esl, mask_s[:, k, :])
                            osl = es[:, 512 + k * QB:512 + (k + 1) * QB]
                            nc.vector.tensor_mul(osl, osl,
                                                 mask_s[:, 2 + k, :])
                    es_t[p] = es

                first_p = {}
                last_p = {}
                for p, (i, g) in enumerate(pairs):
                    first_p.setdefault(i, p)
                    last_p[i] = p

                def emit_pv(p):
                    i, g = pairs[p]
                    if p == first_p[i]:
                        po_t[i] = psum_o.tile([H + 2, QB], f32, tag="po", name=_nm("po"))
                    es = es_t.pop(p)
                    for k in range(2):
                        j = 2 * g + k
                        nc.tensor.matmul(
                            po_t[i][:], vp[j // 4][:, j % 4, 0, 0:H + 2],
                            es[:, k * QB:(k + 1) * QB],
                            start=(p == first_p[i] and k == 0), stop=False)
                        nc.tensor.matmul(
                            po_t[i][:], vp[j // 4][:, j % 4, 1, 0:H + 2],
                            es[:, 512 + k * QB:512 + (k + 1) * QB],
                            start=False, stop=(p == last_p[i] and k == 1))
                    if p == last_p[i]:
                        po = po_t.pop(i)
                        ot = smallp.tile([H + 2, QB], bf16, tag="ot", name=_nm("ot"))
                        nc.vector.tensor_copy(ot[:], po[:])
                        ot_t[i] = ot

                def emit_out(i):
                    # transpose out^T back, divide by the denominator
                    # column, store
                    ot = ot_t.pop(i)
                    for h2 in range(2):
                        pt2 = psum_t.tile([P, H + 2], bf16, tag="pt", name=_nm("pt2"))
                        nc.tensor.transpose(
                            pt2[:], ot[0:H + 2, h2 * P:(h2 + 1) * P],
                            iden[0:H + 2, 0:H + 2])
                        rc = smallp.tile([P, 1], f32, tag="rc", name=_nm("rc"))
                        nc.vector.reciprocal(rc[:], pt2[:, H:H + 1])
                        nc.vector.tensor_scalar_mul(ys[:, 2 * i + h2, :],
                                                    pt2[:, 0:H], rc[:])
                    if i % 2 == 1 and not nodma:
                        u0 = (i - 1) * 2
                        dst = y_d[u0 * P:(u0 + 4) * P, :].rearrange(
                            "(u p) c -> p u c", u=4)
                        hw_eng[i // 2].dma_start(dst, ys[:, u0:u0 + 4, :])

                # scores run 2 groups ahead of PV so the ScalarE exp (and
                # the DVE mask multiplies) never gate a PV matmul
                LA = 2
                for p in range(len(pairs)):
                    if p == 7:
                        emit_vprime(S, 1)
                    emit_scores(p)
                    if p >= LA:
                        emit_pv(p - LA)
                        if p - LA == last_p[pairs[p - LA][0]]:
                            emit_out(pairs[p - LA][0])
                for p in range(len(pairs) - LA, len(pairs)):
                    emit_pv(p)
                    if p == last_p[pairs[p][0]]:
                        emit_out(pairs[p][0])
                for c in interleave:
                    c()

            # ---- 3-deep pipeline ----
            # body u: [prefetch(u+2) early | front-compute(u+1) |
            #          attn(u) | prefetch(u+2) late]; loads lead their
            # consumers by two bodies, projections+exchange by one.
            import concourse.mybir as mybir_
            if one_shot:
                emit_front_loads(sets[0])
                emit_front(sets[0])
                emit_attn(sets[0])
            else:
                emit_front_loads(sets[0])  # iter 0 inputs
                emit_front_loads(sets[1])  # iter 1 inputs
                emit_front(sets[0])        # front-compute(0)
                with (tc.For_i(0, trip, 1,
                               hint_engines=(mybir_.EngineType.PE,
                                             mybir_.EngineType.SP,
                                             mybir_.EngineType.Activation,
                                             mybir_.EngineType.DVE,
                                             mybir_.EngineType.Pool))
                      if trip > 1 else contextlib.nullcontext()):
                    for u in range(U):
                        emit_prefetch_early(sets[u % NSET])
                        emit_vprime(sets[u % NSET], 0)
                        emit_front(sets[(u + 1) % NSET])
                        emit_attn(sets[u % NSET], vprime0_done=True)
                emit_attn(sets[(U - 1) % NSET])

    nc.compile()
    return nc


def _make_masks():
    i = np.arange(P)[:, None]
    j = np.arange(QB)[None, :]
    ma = (i <= j).astype(np.float32)
    mb = (i + P <= j).astype(np.float32)
    return ma, mb


def make_in_maps(x, Wq, Wk, Wv):
    """Per-core input dicts. Core 2*b + t owns query chunks {t, t+2, t+4, t+6}.

    kvv layout after the pairwise AllGather is global-fixed: even-chunk
    K^T units on partitions 0:64, odd-chunk on 64:128; q-block i masks
    its last even pair (t=0: diagonal, t=1: ones) and last odd pair
    (t=0: zeros, t=1: diagonal).
    """
    import ml_dtypes
    bf16 = ml_dtypes.bfloat16

    wkv = np.concatenate([Wk, Wv], axis=1).astype(np.float32)
    wkv = np.ascontiguousarray(
        wkv.reshape(NCB, P, P).transpose(1, 0, 2)).astype(bf16)
    wq = np.asarray(Wq, np.float32).reshape(NCB, P, H).transpose(1, 0, 2)
    wq2 = np.ascontiguousarray(
        np.concatenate([wq, wq], axis=2)).astype(bf16)
    iden = np.eye(P, dtype=np.float32).astype(bf16)
    ma, mb = _make_masks()
    ones = np.ones((P, QB), np.float32)
    zeros = np.zeros((P, QB), np.float32)
    xc = np.asarray(x, np.float32).reshape(B, 8, QB, C)
    in_maps = []
    for core in range(NCORES):
        b, t = divmod(core, 2)
        own = [2 * k + t for k in range(4)]
        xq = xc[b, own].reshape(T // 2, C)
        # host-side transpose: xqT[p, h, cb, t'] = xq[h*512+t', cb*128+p]
        xqT = np.ascontiguousarray(
            xq.T.reshape(NCB, P, 2, 512).transpose(1, 2, 0, 3)).astype(bf16)
        if t == 0:
            msk = np.stack([ma, mb, zeros, zeros], axis=1)
        else:
            msk = np.stack([ones, ones, ma, mb], axis=1)
        in_maps.append({
            "xqT": xqT, "wq2": wq2, "wkv": wkv, "iden": iden,
            "mask": np.ascontiguousarray(msk).astype(bf16),
        })
    return in_maps


def assemble(results):
    y = np.empty((B, T, H), np.float32)
    for core in range(NCORES):
        b, t = divmod(core, 2)
        yc = results[core]["y"]
        for i in range(4):
            g = 2 * i + t
            y[b, g * QB:(g + 1) * QB, :] = yc[i * QB:(i + 1) * QB, :]
    return y


def kernel(x, Wq, Wk, Wv):
    from concourse.bass_utils import run_bass_kernel_spmd
    if "nc" not in _CACHE:
        _CACHE["nc"] = _build_program()
    nc = _CACHE["nc"]
    in_maps = make_in_maps(x, Wq, Wk, Wv)
    try:
        res = run_bass_kernel_spmd(nc, in_maps, list(range(NCORES)))
    except Exception:
        # transient NRT device errors on a cold first dispatch recover on
        # retry
        res = run_bass_kernel_spmd(nc, in_maps, list(range(NCORES)))
    return assemble(res.results)
